# revision 3
# baseline (speedup 1.0000x reference)
"""Trainium2 Bass kernel for nn_Attention_84585085927925.

Reference (per batch element b, all fp32):
    qkv = x @ w_qkv.T                       # [N, 3D]
    q, k, v = split(qkv)                    # heads: [N, H, 64]
    attn = sqrt(64) * q @ k.T               # NO softmax
    out  = attn @ v
    out  = out @ w_fc.T + b_fc

Because there is no softmax, attention is associative:
    out_h = (s*q_h) @ (k_h.T @ v_h)         # [N,64] @ [64,64]
which drops the attention cost from O(N^2 d) to O(N d^2).

Sharding: data-parallel over batch, one batch element per NeuronCore
(B == 8 == n_cores). No collectives.

Per-core dataflow (e = output-feature axis, d = input-feature axis):
    xT   [768,1024]  x.T, host-transposed
    qT   = (s*w_q).T-stationary @ xT        -> [768,1024] (transposed layout)
    k    = xT-stationary @ w_k.T            -> [1024,768] (natural layout)
    v    = xT-stationary @ w_v.T            -> [1024,768]
    G    = k_pair.T @ v_pair per head pair  -> block-diag [128,128] per pair
    aoT  = G2.T-stationary @ qT per pair    -> [768,1024] (= attn-out.T)
    outT = w_fc.T-stationary @ aoT + b_fc   -> [768,1024]
Host transposes outT back.

Matmuls run in float32r (fp32 data, reduced-precision PE mode, 4x faster
than plain fp32, ~1.5e-4 relative error).
"""

import numpy as np

import concourse.bass as bass  # noqa: F401  (registers engine namespaces)
import concourse.mybir as mybir
import concourse.tile as tile
from concourse import bacc, bass_utils

F32 = mybir.dt.float32
F32R = mybir.dt.float32r

B, N, D, H = 8, 1024, 768, 12
HD = D // H            # 64
SCALE = float(np.sqrt(HD))
DT = D // 128          # 6  d-tiles
ET = D // 128          # 6  e-tiles
NT = N // 128          # 8  n(token)-tiles
NC2 = N // 512         # 2  512-wide token chunks
ECH = 384              # e-chunk for k/v projections (fits one PSUM bank)
NPAIR = H // 2         # 6 head pairs


def _build_program():
    nc = bacc.Bacc(
        trn_type="TRN2", target_bir_lowering=False, debug=False, num_devices=B
    )
    xT_d = nc.dram_tensor("xT", [D, N], F32, kind="ExternalInput").ap()
    wqT_d = nc.dram_tensor("wqT", [D, D], F32, kind="ExternalInput").ap()
    wkT_d = nc.dram_tensor("wkT", [D, D], F32, kind="ExternalInput").ap()
    wvT_d = nc.dram_tensor("wvT", [D, D], F32, kind="ExternalInput").ap()
    wfcT_d = nc.dram_tensor("wfcT", [D, D], F32, kind="ExternalInput").ap()
    bfc_d = nc.dram_tensor("bfc", [D], F32, kind="ExternalInput").ap()
    outT_d = nc.dram_tensor("outT", [D, N], F32, kind="ExternalOutput").ap()

    with tile.TileContext(nc) as tc:
        with tc.tile_pool(name="big", bufs=1) as big, \
             tc.tile_pool(name="wsp", bufs=3) as wsp, \
             tc.tile_pool(name="outsp", bufs=2) as outsp, \
             tc.tile_pool(name="ps", bufs=6, space="PSUM") as ps, \
             tc.tile_pool(name="psg", bufs=2, space="PSUM") as psg:

            xT_sb = big.tile([128, DT, N], F32R, name="xT_sb")
            qT_sb = big.tile([128, ET, N], F32R, name="qT_sb")
            k_sb = big.tile([128, NT, D], F32R, name="k_sb")
            v_sb = big.tile([128, NT, D], F32R, name="v_sb")
            ao_sb = big.tile([128, DT, N], F32R, name="ao_sb")
            g2_sb = big.tile([128, NPAIR, 128], F32R, name="g2_sb")
            bias_sb = big.tile([128, ET], F32, name="bias_sb")

            nc.sync.dma_start(bias_sb[:], bfc_d.rearrange("(o p) -> p o", p=128))
            xT_r = xT_d.rearrange("(o p) n -> p o n", p=128).bitcast(F32R)
            for half in range(2):
                sl = slice(half * 512, half * 512 + 512)
                nc.sync.dma_start(xT_sb[:, :, sl], xT_r[:, :, sl])

            # ---- q.T projection: lhsT = wqT tile [d,e], rhs = xT chunk ----
            wq_r = wqT_d.rearrange("(o p) e -> p o e", p=128).bitcast(F32R)
            for et in range(ET):
                wq_t = wsp.tile([128, DT, 128], F32R, tag="w128", name="wq_t")
                nc.sync.dma_start(wq_t[:], wq_r[:, :, et * 128:(et + 1) * 128])
                for ic in range(NC2):
                    pt = ps.tile([128, 512], F32, tag="ps", name="pt_q")
                    for dt in range(DT):
                        nc.tensor.matmul(
                            pt[:],
                            wq_t[:, dt, :],
                            xT_sb[:, dt, ic * 512:(ic + 1) * 512],
                            start=(dt == 0), stop=(dt == DT - 1),
                        )
                    nc.vector.tensor_copy(
                        qT_sb[:, et, ic * 512:(ic + 1) * 512], pt[:]
                    )

            # ---- k and v projections (natural layout) ----
            # lhsT = xT tile [d, n-tile] (stationary), rhs = w{k,v}T [d, e-chunk]
            for name, w_d, dst in (("k", wkT_d, k_sb), ("v", wvT_d, v_sb)):
                w_r = w_d.rearrange("(o p) e -> p o e", p=128).bitcast(F32R)
                for ec in range(D // ECH):
                    w_t = wsp.tile([128, DT, ECH], F32R, tag="w384",
                                   name=f"w{name}_t")
                    nc.sync.dma_start(
                        w_t[:], w_r[:, :, ec * ECH:(ec + 1) * ECH]
                    )
                    for nt in range(NT):
                        pt = ps.tile([128, ECH], F32, tag="ps",
                                     name=f"pt_{name}")
                        for dt in range(DT):
                            nc.tensor.matmul(
                                pt[:],
                                xT_sb[:, dt, nt * 128:(nt + 1) * 128],
                                w_t[:, dt, :],
                                start=(dt == 0), stop=(dt == DT - 1),
                            )
                        nc.vector.tensor_copy(
                            dst[:, nt, ec * ECH:(ec + 1) * ECH], pt[:]
                        )

            # ---- G = k.T @ v per head pair, stored block-diagonal ----
            # One matmul per (pair, j-tile): [128(j),128(dk pair)].T @
            # [128(j),128(dv pair)] accumulated over j-tiles. The diagonal
            # 64x64 blocks are G_2t and G_2t+1; off-diagonal blocks are
            # cross-head terms, discarded by copying only the diagonal.
            for t in range(NPAIR):
                gp = psg.tile([128, 128], F32, tag="psg", name="gp")
                for jt in range(NT):
                    nc.tensor.matmul(
                        gp[:],
                        k_sb[:, jt, t * 128:(t + 1) * 128],
                        v_sb[:, jt, t * 128:(t + 1) * 128],
                        start=(jt == 0), stop=(jt == NT - 1),
                    )
                nc.vector.tensor_scalar_mul(g2_sb[:, t, :], gp[:], 0.0)
                nc.vector.tensor_copy(g2_sb[0:64, t, 0:64], gp[0:64, 0:64])
                nc.vector.tensor_copy(g2_sb[64:128, t, 64:128],
                                      gp[64:128, 64:128])

            # ---- attn-out.T = G2.T @ qT per pair ----
            for t in range(NPAIR):
                for ic in range(NC2):
                    pt = ps.tile([128, 512], F32, tag="ps", name="pt_ao")
                    nc.tensor.matmul(
                        pt[:],
                        g2_sb[:, t, :],
                        qT_sb[:, t, ic * 512:(ic + 1) * 512],
                        start=True, stop=True,
                    )
                    nc.vector.tensor_copy(
                        ao_sb[:, t, ic * 512:(ic + 1) * 512], pt[:]
                    )

            # ---- fc: outT = wfcT.T-stationary @ aoT + bias ----
            wfc_r = wfcT_d.rearrange("(o p) e -> p o e", p=128).bitcast(F32R)
            for et in range(ET):
                wfc_t = wsp.tile([128, DT, 128], F32R, tag="w128",
                                 name="wfc_t")
                nc.sync.dma_start(wfc_t[:], wfc_r[:, :, et * 128:(et + 1) * 128])
                for ic in range(NC2):
                    pt = ps.tile([128, 512], F32, tag="ps", name="pt_fc")
                    for dt in range(DT):
                        nc.tensor.matmul(
                            pt[:],
                            wfc_t[:, dt, :],
                            ao_sb[:, dt, ic * 512:(ic + 1) * 512],
                            start=(dt == 0), stop=(dt == DT - 1),
                        )
                    ot = outsp.tile([128, 512], F32, tag="ot", name="ot")
                    nc.scalar.add(ot[:], pt[:], bias_sb[:, et:et + 1])
                    nc.sync.dma_start(
                        outT_d[et * 128:(et + 1) * 128,
                               ic * 512:(ic + 1) * 512],
                        ot[:],
                    )

    nc.compile()
    return nc


_NC_CACHE = None
LAST_EXEC_NS = None


def kernel(x, w_qkv, w_fc, b_fc, _trace=False):
    global _NC_CACHE, LAST_EXEC_NS
    x = np.asarray(x, dtype=np.float32)
    w_qkv = np.asarray(w_qkv, dtype=np.float32)
    w_fc = np.asarray(w_fc, dtype=np.float32)
    b_fc = np.asarray(b_fc, dtype=np.float32)

    if _NC_CACHE is None:
        _NC_CACHE = _build_program()
    nc = _NC_CACHE

    wqT = np.ascontiguousarray((SCALE * w_qkv[:D]).T)
    wkT = np.ascontiguousarray(w_qkv[D:2 * D].T)
    wvT = np.ascontiguousarray(w_qkv[2 * D:].T)
    wfcT = np.ascontiguousarray(w_fc.T)

    in_maps = []
    for b in range(B):
        in_maps.append({
            "xT": np.ascontiguousarray(x[b].T),
            "wqT": wqT, "wkT": wkT, "wvT": wvT, "wfcT": wfcT,
            "bfc": b_fc,
        })

    res = bass_utils.run_bass_kernel_spmd(
        nc, in_maps, core_ids=list(range(B)), trace=_trace
    )
    LAST_EXEC_NS = res.exec_time_ns
    global LAST_RES
    LAST_RES = res
    out = np.stack([res.results[b]["outT"].T for b in range(B)])
    return np.ascontiguousarray(out.astype(np.float32))


# revision 5
# speedup vs baseline: 1.0217x; 1.0217x over previous
"""Trainium2 Bass kernel for nn_Attention_84585085927925.

Reference (per batch element b, all fp32):
    qkv = x @ w_qkv.T                       # [N, 3D]
    q, k, v = split(qkv)                    # heads: [N, H, 64]
    attn = sqrt(64) * q @ k.T               # NO softmax
    out  = attn @ v
    out  = out @ w_fc.T + b_fc

Because there is no softmax, attention is associative:
    out_h = (s*q_h) @ (k_h.T @ v_h)         # [N,64] @ [64,64]
which drops the attention cost from O(N^2 d) to O(N d^2).

Sharding: data-parallel over batch, one batch element per NeuronCore
(B == 8 == n_cores). No collectives.

Per-core dataflow (e = output-feature axis, d = input-feature axis):
    xT   [768,1024]  x.T, host-transposed
    qT   = (s*w_q).T-stationary @ xT        -> [768,1024] (transposed layout)
    k    = xT-stationary @ w_k.T            -> [1024,768] (natural layout)
    v    = xT-stationary @ w_v.T            -> [1024,768]
    G    = k_pair.T @ v_pair per head pair  -> block-diag [128,128] per pair
    aoT  = G2.T-stationary @ qT per pair    -> [768,1024] (= attn-out.T)
    outT = w_fc.T-stationary @ aoT + b_fc   -> [768,1024]
Host transposes outT back.

Matmuls run in float32r (fp32 data, reduced-precision PE mode, 4x faster
than plain fp32, ~1.5e-4 relative error).
"""

import numpy as np

import concourse.bass as bass  # noqa: F401  (registers engine namespaces)
import concourse.mybir as mybir
import concourse.tile as tile
from concourse import bacc, bass_utils

F32 = mybir.dt.float32
F32R = mybir.dt.float32r

B, N, D, H = 8, 1024, 768, 12
HD = D // H            # 64
SCALE = float(np.sqrt(HD))
DT = D // 128          # 6  d-tiles
ET = D // 128          # 6  e-tiles
NT = N // 128          # 8  n(token)-tiles
NC2 = N // 512         # 2  512-wide token chunks
ECH = 384              # e-chunk for k/v projections (fits one PSUM bank)
NPAIR = H // 2         # 6 head pairs


def _build_program():
    nc = bacc.Bacc(
        trn_type="TRN2", target_bir_lowering=False, debug=False, num_devices=B
    )
    xT_d = nc.dram_tensor("xT", [D, N], F32, kind="ExternalInput").ap()
    wqT_d = nc.dram_tensor("wqT", [D, D], F32, kind="ExternalInput").ap()
    wkT_d = nc.dram_tensor("wkT", [D, D], F32, kind="ExternalInput").ap()
    wvT_d = nc.dram_tensor("wvT", [D, D], F32, kind="ExternalInput").ap()
    wfcT_d = nc.dram_tensor("wfcT", [D, D], F32, kind="ExternalInput").ap()
    bfc_d = nc.dram_tensor("bfc", [D], F32, kind="ExternalInput").ap()
    outT_d = nc.dram_tensor("outT", [D, N], F32, kind="ExternalOutput").ap()

    with tile.TileContext(nc) as tc:
        with tc.tile_pool(name="big", bufs=1) as big, \
             tc.tile_pool(name="wsp", bufs=3) as wsp, \
             tc.tile_pool(name="outsp", bufs=2) as outsp, \
             tc.tile_pool(name="ps", bufs=6, space="PSUM") as ps, \
             tc.tile_pool(name="psg", bufs=2, space="PSUM") as psg:

            xT_sb = big.tile([128, DT, N], F32R, name="xT_sb")
            qT_sb = big.tile([128, ET, N], F32R, name="qT_sb")
            k_sb = big.tile([128, NT, D], F32R, name="k_sb")
            v_sb = big.tile([128, NT, D], F32R, name="v_sb")
            ao_sb = big.tile([128, DT, N], F32R, name="ao_sb")
            g2_sb = big.tile([128, NPAIR, 128], F32R, name="g2_sb")
            bias_sb = big.tile([128, ET], F32, name="bias_sb")

            # Critical-path DMAs first, split small so they spread across
            # HW-DGE queues: wq(et=0) halves, then xT n-half 0 per d-tile.
            wq_r = wqT_d.rearrange("(o p) e -> p o e", p=128).bitcast(F32R)
            xT_r = xT_d.rearrange("(o p) n -> p o n", p=128).bitcast(F32R)
            wq_tiles = []
            for et in range(ET):
                wq_t = wsp.tile([128, DT, 128], F32R, tag="w128",
                                name=f"wq_t{et}", uniquify=False)
                wq_tiles.append(wq_t)
            for dh in range(2):
                dsl = slice(dh * 3, dh * 3 + 3)
                nc.sync.dma_start(wq_tiles[0][:, dsl, :],
                                  wq_r[:, dsl, 0:128])
            for half in range(2):
                nsl = slice(half * 512, half * 512 + 512)
                for dt in range(DT):
                    nc.sync.dma_start(xT_sb[:, dt, nsl], xT_r[:, dt, nsl])
                if half == 0:
                    for dh in range(2):
                        dsl = slice(dh * 3, dh * 3 + 3)
                        nc.sync.dma_start(wq_tiles[1][:, dsl, :],
                                          wq_r[:, dsl, 128:256])
            nc.sync.dma_start(bias_sb[:], bfc_d.rearrange("(o p) -> p o", p=128))

            # ---- q.T projection: lhsT = wqT tile [d,e], rhs = xT chunk ----
            for et in range(ET):
                wq_t = wq_tiles[et]
                if et >= 2:
                    nc.sync.dma_start(wq_t[:], wq_r[:, :, et * 128:(et + 1) * 128])
                for ic in range(NC2):
                    pt = ps.tile([128, 512], F32, tag="ps", name="pt_q")
                    for dt in range(DT):
                        nc.tensor.matmul(
                            pt[:],
                            wq_t[:, dt, :],
                            xT_sb[:, dt, ic * 512:(ic + 1) * 512],
                            start=(dt == 0), stop=(dt == DT - 1),
                        )
                    nc.vector.tensor_copy(
                        qT_sb[:, et, ic * 512:(ic + 1) * 512], pt[:]
                    )

            # ---- k and v projections (natural layout) ----
            # lhsT = xT tile [d, n-tile] (stationary), rhs = w{k,v}T [d, e-chunk]
            for name, w_d, dst in (("k", wkT_d, k_sb), ("v", wvT_d, v_sb)):
                w_r = w_d.rearrange("(o p) e -> p o e", p=128).bitcast(F32R)
                for ec in range(D // ECH):
                    w_t = wsp.tile([128, DT, ECH], F32R, tag="w384",
                                   name=f"w{name}_t")
                    for dt in range(DT):
                        nc.sync.dma_start(
                            w_t[:, dt, :],
                            w_r[:, dt, ec * ECH:(ec + 1) * ECH],
                        )
                    for nt in range(NT):
                        pt = ps.tile([128, ECH], F32, tag="ps",
                                     name=f"pt_{name}")
                        for dt in range(DT):
                            nc.tensor.matmul(
                                pt[:],
                                xT_sb[:, dt, nt * 128:(nt + 1) * 128],
                                w_t[:, dt, :],
                                start=(dt == 0), stop=(dt == DT - 1),
                            )
                        nc.vector.tensor_copy(
                            dst[:, nt, ec * ECH:(ec + 1) * ECH], pt[:]
                        )

            # ---- G = k.T @ v per head pair, stored block-diagonal ----
            # One matmul per (pair, j-tile): [128(j),128(dk pair)].T @
            # [128(j),128(dv pair)] accumulated over j-tiles. The diagonal
            # 64x64 blocks are G_2t and G_2t+1; off-diagonal blocks are
            # cross-head terms, discarded by copying only the diagonal.
            for t in range(NPAIR):
                gp = psg.tile([128, 128], F32, tag="psg", name="gp")
                for jt in range(NT):
                    nc.tensor.matmul(
                        gp[:],
                        k_sb[:, jt, t * 128:(t + 1) * 128],
                        v_sb[:, jt, t * 128:(t + 1) * 128],
                        start=(jt == 0), stop=(jt == NT - 1),
                    )
                nc.vector.tensor_scalar_mul(g2_sb[:, t, :], gp[:], 0.0)
                nc.vector.tensor_copy(g2_sb[0:64, t, 0:64], gp[0:64, 0:64])
                nc.vector.tensor_copy(g2_sb[64:128, t, 64:128],
                                      gp[64:128, 64:128])

            # ---- attn-out.T = G2.T @ qT per pair ----
            for t in range(NPAIR):
                for ic in range(NC2):
                    pt = ps.tile([128, 512], F32, tag="ps", name="pt_ao")
                    nc.tensor.matmul(
                        pt[:],
                        g2_sb[:, t, :],
                        qT_sb[:, t, ic * 512:(ic + 1) * 512],
                        start=True, stop=True,
                    )
                    nc.vector.tensor_copy(
                        ao_sb[:, t, ic * 512:(ic + 1) * 512], pt[:]
                    )

            # ---- fc: outT = wfcT.T-stationary @ aoT + bias ----
            wfc_r = wfcT_d.rearrange("(o p) e -> p o e", p=128).bitcast(F32R)
            for et in range(ET):
                wfc_t = wsp.tile([128, DT, 128], F32R, tag="w128",
                                 name="wfc_t")
                nc.sync.dma_start(wfc_t[:], wfc_r[:, :, et * 128:(et + 1) * 128])
                for ic in range(NC2):
                    pt = ps.tile([128, 512], F32, tag="ps", name="pt_fc")
                    for dt in range(DT):
                        nc.tensor.matmul(
                            pt[:],
                            wfc_t[:, dt, :],
                            ao_sb[:, dt, ic * 512:(ic + 1) * 512],
                            start=(dt == 0), stop=(dt == DT - 1),
                        )
                    ot = outsp.tile([128, 512], F32, tag="ot", name="ot")
                    nc.scalar.add(ot[:], pt[:], bias_sb[:, et:et + 1])
                    nc.sync.dma_start(
                        outT_d[et * 128:(et + 1) * 128,
                               ic * 512:(ic + 1) * 512],
                        ot[:],
                    )

    nc.compile()
    return nc


_NC_CACHE = None
LAST_EXEC_NS = None


def kernel(x, w_qkv, w_fc, b_fc, _trace=False):
    global _NC_CACHE, LAST_EXEC_NS
    x = np.asarray(x, dtype=np.float32)
    w_qkv = np.asarray(w_qkv, dtype=np.float32)
    w_fc = np.asarray(w_fc, dtype=np.float32)
    b_fc = np.asarray(b_fc, dtype=np.float32)

    if _NC_CACHE is None:
        _NC_CACHE = _build_program()
    nc = _NC_CACHE

    wqT = np.ascontiguousarray((SCALE * w_qkv[:D]).T)
    wkT = np.ascontiguousarray(w_qkv[D:2 * D].T)
    wvT = np.ascontiguousarray(w_qkv[2 * D:].T)
    wfcT = np.ascontiguousarray(w_fc.T)

    in_maps = []
    for b in range(B):
        in_maps.append({
            "xT": np.ascontiguousarray(x[b].T),
            "wqT": wqT, "wkT": wkT, "wvT": wvT, "wfcT": wfcT,
            "bfc": b_fc,
        })

    res = bass_utils.run_bass_kernel_spmd(
        nc, in_maps, core_ids=list(range(B)), trace=_trace
    )
    LAST_EXEC_NS = res.exec_time_ns
    global LAST_RES
    LAST_RES = res
    out = np.stack([res.results[b]["outT"].T for b in range(B)])
    return np.ascontiguousarray(out.astype(np.float32))


# revision 8
# speedup vs baseline: 1.0523x; 1.0299x over previous
"""Trainium2 Bass kernel for nn_Attention_84585085927925.

Reference (per batch element b, all fp32):
    qkv = x @ w_qkv.T                       # [N, 3D]
    q, k, v = split(qkv)                    # heads: [N, H, 64]
    attn = sqrt(64) * q @ k.T               # NO softmax
    out  = attn @ v
    out  = out @ w_fc.T + b_fc

Because there is no softmax, attention is associative:
    out_h = (s*q_h) @ (k_h.T @ v_h)         # [N,64] @ [64,64]
which drops the attention cost from O(N^2 d) to O(N d^2).

Sharding: data-parallel over batch, one batch element per NeuronCore
(B == 8 == n_cores). No collectives.

Per-core dataflow (e = output-feature axis, d = input-feature axis):
    xT   [768,1024]  x.T, host-transposed
    qT   = (s*w_q).T-stationary @ xT        -> [768,1024] (transposed layout)
    k    = xT-stationary @ w_k.T            -> [1024,768] (natural layout)
    v    = xT-stationary @ w_v.T            -> [1024,768]
    G    = k_pair.T @ v_pair per head pair  -> block-diag [128,128] per pair
    aoT  = G2.T-stationary @ qT per pair    -> [768,1024] (= attn-out.T)
    outT = w_fc.T-stationary @ aoT + b_fc   -> [768,1024]
Host transposes outT back.

Matmuls run in float32r (fp32 data, reduced-precision PE mode, 4x faster
than plain fp32, ~1.5e-4 relative error).
"""

import numpy as np

import concourse.bass as bass  # noqa: F401  (registers engine namespaces)
import concourse.mybir as mybir
import concourse.tile as tile
from concourse import bacc, bass_utils

F32 = mybir.dt.float32
F32R = mybir.dt.float32r

B, N, D, H = 8, 1024, 768, 12
HD = D // H            # 64
SCALE = float(np.sqrt(HD))
DT = D // 128          # 6  d-tiles
ET = D // 128          # 6  e-tiles
NT = N // 128          # 8  n(token)-tiles
NC2 = N // 512         # 2  512-wide token chunks
ECH = 384              # e-chunk for k/v projections (fits one PSUM bank)
NPAIR = H // 2         # 6 head pairs


def _build_program():
    nc = bacc.Bacc(
        trn_type="TRN2", target_bir_lowering=False, debug=False, num_devices=B
    )
    xT_d = nc.dram_tensor("xT", [D, N], F32, kind="ExternalInput").ap()
    wqT_d = nc.dram_tensor("wqT", [D, D], F32, kind="ExternalInput").ap()
    wkT_d = nc.dram_tensor("wkT", [D, D], F32, kind="ExternalInput").ap()
    wvT_d = nc.dram_tensor("wvT", [D, D], F32, kind="ExternalInput").ap()
    wfcT_d = nc.dram_tensor("wfcT", [D, D], F32, kind="ExternalInput").ap()
    bfc_d = nc.dram_tensor("bfc", [D], F32, kind="ExternalInput").ap()
    outT_d = nc.dram_tensor("outT", [D, N], F32, kind="ExternalOutput").ap()

    with tile.TileContext(nc) as tc:
        with tc.tile_pool(name="big", bufs=1) as big, \
             tc.tile_pool(name="wsp", bufs=3) as wsp, \
             tc.tile_pool(name="outsp", bufs=2) as outsp, \
             tc.tile_pool(name="ps", bufs=6, space="PSUM") as ps, \
             tc.tile_pool(name="psg", bufs=2, space="PSUM") as psg:

            xT_sb = big.tile([128, DT, N], F32R, name="xT_sb")
            qT_sb = big.tile([128, ET, N], F32R, name="qT_sb")
            k_sb = big.tile([128, NT, D], F32R, name="k_sb")
            v_sb = big.tile([128, NT, D], F32R, name="v_sb")
            ao_sb = big.tile([128, DT, N], F32R, name="ao_sb")
            g2_sb = big.tile([128, NPAIR, 128], F32R, name="g2_sb")
            bias_sb = big.tile([128, ET], F32, name="bias_sb")

            # Critical-path DMAs first, split small so they spread across
            # HW-DGE queues: wq(et=0) halves, then xT n-half 0 per d-tile.
            wq_r = wqT_d.rearrange("(o p) e -> p o e", p=128).bitcast(F32R)
            xT_r = xT_d.rearrange("(o p) n -> p o n", p=128).bitcast(F32R)
            wq_tiles = []
            for et in range(ET):
                wq_t = wsp.tile([128, DT, 128], F32R, tag="w128", bufs=7,
                                name=f"wq_t{et}", uniquify=False)
                wq_tiles.append(wq_t)
            # first-needed data first: wq0 halves, xT n-half 0, then the rest
            for dh in range(2):
                dsl = slice(dh * 3, dh * 3 + 3)
                nc.sync.dma_start(wq_tiles[0][:, dsl, :], wq_r[:, dsl, 0:128])
            for dt in range(DT):
                nc.sync.dma_start(xT_sb[:, dt, 0:512], xT_r[:, dt, 0:512])
            for et in range(1, ET):
                nc.sync.dma_start(wq_tiles[et][:],
                                  wq_r[:, :, et * 128:(et + 1) * 128])
            for dt in range(DT):
                nc.sync.dma_start(xT_sb[:, dt, 512:1024], xT_r[:, dt, 512:1024])
            nc.sync.dma_start(bias_sb[:], bfc_d.rearrange("(o p) -> p o", p=128))

            # ---- q.T projection: lhsT = wqT tile [d,e], rhs = xT chunk ----
            for ic in range(NC2):
                for et in range(ET):
                    wq_t = wq_tiles[et]
                    pt = ps.tile([128, 512], F32, tag="ps", name="pt_q")
                    for dt in range(DT):
                        nc.tensor.matmul(
                            pt[:],
                            wq_t[:, dt, :],
                            xT_sb[:, dt, ic * 512:(ic + 1) * 512],
                            start=(dt == 0), stop=(dt == DT - 1),
                        )
                    nc.vector.tensor_copy(
                        qT_sb[:, et, ic * 512:(ic + 1) * 512], pt[:]
                    )

            # ---- k and v projections (natural layout) ----
            # lhsT = xT tile [d, n-tile] (stationary), rhs = w{k,v}T [d, e-chunk]
            for name, w_d, dst in (("k", wkT_d, k_sb), ("v", wvT_d, v_sb)):
                w_r = w_d.rearrange("(o p) e -> p o e", p=128).bitcast(F32R)
                for ec in range(D // ECH):
                    w_t = wsp.tile([128, DT, ECH], F32R, tag="w384",
                                   name=f"w{name}_t")
                    for dt in range(DT):
                        nc.sync.dma_start(
                            w_t[:, dt, :],
                            w_r[:, dt, ec * ECH:(ec + 1) * ECH],
                        )
                    for nt in range(NT):
                        pt = ps.tile([128, ECH], F32, tag="ps",
                                     name=f"pt_{name}")
                        for dt in range(DT):
                            nc.tensor.matmul(
                                pt[:],
                                xT_sb[:, dt, nt * 128:(nt + 1) * 128],
                                w_t[:, dt, :],
                                start=(dt == 0), stop=(dt == DT - 1),
                            )
                        nc.vector.tensor_copy(
                            dst[:, nt, ec * ECH:(ec + 1) * ECH], pt[:]
                        )

            # ---- G = k.T @ v per head pair, stored block-diagonal ----
            # One matmul per (pair, j-tile): [128(j),128(dk pair)].T @
            # [128(j),128(dv pair)] accumulated over j-tiles. The diagonal
            # 64x64 blocks are G_2t and G_2t+1; off-diagonal blocks are
            # cross-head terms, discarded by copying only the diagonal.
            for t in range(NPAIR):
                gp = psg.tile([128, 128], F32, tag="psg", name="gp")
                for jt in range(NT):
                    nc.tensor.matmul(
                        gp[:],
                        k_sb[:, jt, t * 128:(t + 1) * 128],
                        v_sb[:, jt, t * 128:(t + 1) * 128],
                        start=(jt == 0), stop=(jt == NT - 1),
                    )
                nc.vector.tensor_scalar_mul(g2_sb[:, t, :], gp[:], 0.0)
                nc.vector.tensor_copy(g2_sb[0:64, t, 0:64], gp[0:64, 0:64])
                nc.vector.tensor_copy(g2_sb[64:128, t, 64:128],
                                      gp[64:128, 64:128])

            # ---- attn-out.T = G2.T @ qT per pair ----
            for t in range(NPAIR):
                for ic in range(NC2):
                    pt = ps.tile([128, 512], F32, tag="ps", name="pt_ao")
                    nc.tensor.matmul(
                        pt[:],
                        g2_sb[:, t, :],
                        qT_sb[:, t, ic * 512:(ic + 1) * 512],
                        start=True, stop=True,
                    )
                    nc.vector.tensor_copy(
                        ao_sb[:, t, ic * 512:(ic + 1) * 512], pt[:]
                    )

            # ---- fc: outT = wfcT.T-stationary @ aoT + bias ----
            wfc_r = wfcT_d.rearrange("(o p) e -> p o e", p=128).bitcast(F32R)
            for et in range(ET):
                wfc_t = wsp.tile([128, DT, 128], F32R, tag="w128", bufs=7,
                                 name="wfc_t")
                nc.sync.dma_start(wfc_t[:], wfc_r[:, :, et * 128:(et + 1) * 128])
                for ic in range(NC2):
                    pt = ps.tile([128, 512], F32, tag="ps", name="pt_fc")
                    for dt in range(DT):
                        nc.tensor.matmul(
                            pt[:],
                            wfc_t[:, dt, :],
                            ao_sb[:, dt, ic * 512:(ic + 1) * 512],
                            start=(dt == 0), stop=(dt == DT - 1),
                        )
                    ot = outsp.tile([128, 512], F32, tag="ot", name="ot")
                    nc.scalar.add(ot[:], pt[:], bias_sb[:, et:et + 1])
                    for ph in range(2):
                        nc.sync.dma_start(
                            outT_d[et * 128 + ph * 64:et * 128 + ph * 64 + 64,
                                   ic * 512:(ic + 1) * 512],
                            ot[ph * 64:ph * 64 + 64, :],
                        )

    nc.compile()
    return nc


_NC_CACHE = None
LAST_EXEC_NS = None


def kernel(x, w_qkv, w_fc, b_fc, _trace=False):
    global _NC_CACHE, LAST_EXEC_NS
    x = np.asarray(x, dtype=np.float32)
    w_qkv = np.asarray(w_qkv, dtype=np.float32)
    w_fc = np.asarray(w_fc, dtype=np.float32)
    b_fc = np.asarray(b_fc, dtype=np.float32)

    if _NC_CACHE is None:
        _NC_CACHE = _build_program()
    nc = _NC_CACHE

    wqT = np.ascontiguousarray((SCALE * w_qkv[:D]).T)
    wkT = np.ascontiguousarray(w_qkv[D:2 * D].T)
    wvT = np.ascontiguousarray(w_qkv[2 * D:].T)
    wfcT = np.ascontiguousarray(w_fc.T)

    in_maps = []
    for b in range(B):
        in_maps.append({
            "xT": np.ascontiguousarray(x[b].T),
            "wqT": wqT, "wkT": wkT, "wvT": wvT, "wfcT": wfcT,
            "bfc": b_fc,
        })

    res = bass_utils.run_bass_kernel_spmd(
        nc, in_maps, core_ids=list(range(B)), trace=_trace
    )
    LAST_EXEC_NS = res.exec_time_ns
    global LAST_RES
    LAST_RES = res
    out = np.stack([res.results[b]["outT"].T for b in range(B)])
    return np.ascontiguousarray(out.astype(np.float32))
